# revision 40
# baseline (speedup 1.0000x reference)
"""Distributed multi-head GAT (encoder + 2 GAT layers) on 8 TRN2 NeuronCores.

Strategy (graph/data parallel, dst-ownership sharding):
  * Nodes are permuted and dealt into 8*NT bins of 128 so each bin (= one
    PSUM dst-tile) has a near-equal number of incoming edges.  Edges live
    with the core that owns their dst node.

  * Each core encodes its shard (obs -> z1, bf16 MLP) and builds a packed
    256B fp8 table row per node: feat[0:128] | el[128:136]; rows are
    AllGathered so any core can gather src rows.
  * Edge pass: per super-tile (SUP bins), batched dma_gather pulls the
    256B rows of that bin-group's edge srcs (low/high table halves on two
    SWDGE queues so int16 indices can address them).  Section sizes are
    baked per (super, bin, half) as the max over cores, and each gather's
    trailing pads are -1 indices the ucode skips (num_idxs_reg = the
    cross-core-uniform real count), which trims descriptor generation on
    GpSimd — the dominant cost — by ~13%.
  * Attention: e = el[src]+er[dst] (er via one-hot matmul), then
    ex = exp(max(e, 0.2e)) (exact leaky-relu+exp, softmax max-subtraction
    skipped: inputs are O(0.1) so exp is safe; matches up to the 1e-9
    epsilon).  Messages feat*ex reduce into the bin's PSUM accumulator via
    a one-hot matmul; the same matmul accumulates the softmax denominator.
  * Epilogue per bin: out = relu(acc_feat)/(acc_z+1e-9) per head; also
    emits the next layer's packed table row + er entries.
"""

import os
import sys
import time

import numpy as np

for _p in ("/opt/trn_rl_repo", "/root/.axon_site/_ro/trn_rl_repo"):
    if os.path.isdir(_p) and _p not in sys.path:
        sys.path.insert(0, _p)

P = 128
NCORES = 8
OBS_D = 256
HID = 512
H = 128          # h_dim
NH = 8           # heads
HD = 16          # head dim
ROWB = 256       # table row (fp8 elems): feat[0:128] | el[128:136] | pad
TAB = H + NH     # rhs row: msg(128) | ex(8)
SUP = 2          # bins per gather super-tile
SPLIT = 32768    # low/high table split for int16 gather indices
PAD_SENTINEL = 200.0
# Supers of gather-descriptor prep-ahead.  Capped at 3: Tile rotates SWDGE
# preps over 8 global DMASW sem lanes (2 preps/super), and a lane's >=16*tick
# wait is only exact when at most one fired gather is outstanding per lane —
# fire-ahead of 4 supers (8 preps, one per lane) is the provable bound, and
# the gather pool's WAR edge (bufs=KA+1) keeps same-lane preps serialized.
K_AHEAD = 3

LAST_INFO = {}


def _ensure_ntff_hook():
    """Register the axon NTFF profile hook if the image's antenv lacks it."""
    try:
        import types

        import antenv
        try:
            from antenv import axon_hooks  # noqa: F401
            return
        except ImportError:
            pass
        m = types.ModuleType("antenv.axon_hooks")
        _h = [None]
        m.set_axon_ntff_profile_hook = lambda hook: _h.__setitem__(0, hook)
        m.get_axon_ntff_profile_hook = lambda: _h[0]
        sys.modules["antenv.axon_hooks"] = m
        antenv.axon_hooks = m
        from trn_agent_boot.trn_boot import _ntff_profile_via_ctypes
        m.set_axon_ntff_profile_hook(
            _ntff_profile_via_ctypes("/opt/axon/libaxon_pjrt.so"))
    except Exception as e:  # profiling is best-effort
        print(f"ntff hook setup failed: {e}")


# ----------------------------------------------------------------------------
# Host-side preprocessing
# ----------------------------------------------------------------------------

def _wrap16(a):
    """[n] -> [128, n/16] int16 in the dma_gather wrapped layout:
    index j lives at partition j%16, col j//16, replicated to all 8 groups."""
    n = a.shape[0]
    w = a.reshape(n // 16, 16).T.astype(np.int16)    # [16, n/16]
    return np.ascontiguousarray(np.tile(w, (8, 1)))


def _host_prepare(src, dst, n_tiles_per_core, split):
    """Balance bins, split edges by src table half, build gather slabs."""
    import ml_dtypes

    NT = n_tiles_per_core
    NPC = NT * P
    NTOT = NCORES * NPC
    NBINS = NCORES * NT
    E = src.shape[0]

    deg = np.bincount(dst, minlength=NTOT).astype(np.int64)
    order = np.argsort(-deg, kind="stable")
    arr = order.reshape(P, NBINS).copy()
    arr[1::2] = arr[1::2, ::-1]
    perm = arr.T.reshape(-1)                       # g -> orig node
    pos = np.empty(NTOT, np.int64)
    pos[perm] = np.arange(NTOT)                    # orig node -> g

    srcg = pos[src]
    dstg = pos[dst]
    # Table rows are laid out chunk-major so each AllGather chunk (the
    # first/second half of every core's shard) lands in one contiguous
    # slice of tab_full; remap gather indices accordingly.
    nch = 2 if NT % 4 == 0 else 1
    if nch == 2:
        half = NPC // 2
        sc, sr = srcg // NPC, srcg % NPC
        k = sr // half
        srcg = k * (NTOT // 2) + sc * half + (sr - k * half)
    binid = dstg // P
    low = srcg < split

    nlo = np.bincount(binid[low], minlength=NBINS)
    nhi = np.bincount(binid[~low], minlength=NBINS)
    TL = max(1, int(np.ceil(nlo.max() / P)))
    TH = max(1, int(np.ceil(nhi.max() / P)))
    TT = TL + TH

    gkey = binid * 2 + (~low).astype(np.int64)
    eorder = np.argsort(gkey, kind="stable")
    counts = np.bincount(gkey, minlength=2 * NBINS)
    starts = np.concatenate([[0], np.cumsum(counts)[:-1]])
    rank = np.arange(E) - starts[gkey[eorder]]
    ghigh = gkey[eorder] % 2
    slot = (gkey[eorder] // 2) * (TT * P) + ghigh * (TL * P) + rank

    ES = NBINS * TT * P
    sg = np.zeros(ES, np.int64)
    isreal = np.zeros(ES, bool)
    dposf = np.full(ES, PAD_SENTINEL, np.float32)
    sg[slot] = srcg[eorder]
    isreal[slot] = True
    dposf[slot] = (dstg[eorder] % P).astype(np.float32)

    # high-section pads must index within the high table half
    sg[~isreal & (np.arange(ES) % (TT * P) >= TL * P)] = split

    sg = sg.reshape(NCORES, NT, TT * P)
    dposf = dposf.reshape(NCORES, NT, TT * P).astype(ml_dtypes.bfloat16)

    # Per-(local bin, half) tile counts: max over cores, so the section
    # layout (baked into the SPMD program) fits every core.  Within a
    # super the second bin's tail pads become -1 (skipped by the gather;
    # num_idxs_reg = the identical-across-cores real+0-pad count).
    nlo2 = nlo.reshape(NCORES, NT)
    nhi2 = nhi.reshape(NCORES, NT)
    TLOb = np.maximum(1, -(-nlo2.max(axis=0) // P))      # [NT]
    THIb = np.maximum(1, -(-nhi2.max(axis=0) // P))

    NSUP = NT // SUP
    meta = dict(l0=[], l1=[], h0=[], h1=[], nrl=[], nrh=[])
    for S in range(NSUP):
        b0, b1 = SUP * S, SUP * S + 1
        l0, l1 = int(TLOb[b0]), int(TLOb[b1])
        h0, h1 = int(THIb[b0]), int(THIb[b1])
        n1l = max(1, int(nlo2[:, b1].max()))
        n1h = max(1, int(nhi2[:, b1].max()))
        meta["l0"].append(l0)
        meta["l1"].append(l1)
        meta["h0"].append(h0)
        meta["h1"].append(h1)
        meta["nrl"].append(l0 * P + n1l)
        meta["nrh"].append(h0 * P + n1h)

    idxlow, idxhigh, dpos, dprow = [], [], [], []
    for c in range(NCORES):
        il, ih, dp, dr = [], [], [], []
        for S in range(NSUP):
            b0, b1 = SUP * S, SUP * S + 1
            l0, l1 = meta["l0"][S], meta["l1"][S]
            h0, h1 = meta["h0"][S], meta["h1"][S]
            lo = np.concatenate([sg[c, b0, :l0 * P], sg[c, b1, :l1 * P]])
            lo[meta["nrl"][S]:] = -1
            hi = np.concatenate([sg[c, b0, TL * P:TL * P + h0 * P],
                                 sg[c, b1, TL * P:TL * P + h1 * P]]) - split
            hi[meta["nrh"][S]:] = -1
            il.append(_wrap16(lo))
            ih.append(_wrap16(hi))
            dpl = np.concatenate(
                [dposf[c, b0, :l0 * P], dposf[c, b1, :l1 * P]]
            ).reshape(l0 + l1, P)
            dph = np.concatenate(
                [dposf[c, b0, TL * P:TL * P + h0 * P],
                 dposf[c, b1, TL * P:TL * P + h1 * P]]
            ).reshape(h0 + h1, P)
            both = np.concatenate([dpl, dph], 0)         # [nst_S, 128]
            dp.append(both.T)                            # [128, nst_S]
            dr.append(both.reshape(-1))                  # slot-order flat
        idxlow.append(np.ascontiguousarray(np.concatenate(il, 1)))
        idxhigh.append(np.ascontiguousarray(np.concatenate(ih, 1)))
        dpos.append(np.ascontiguousarray(
            np.concatenate(dp, 1).astype(ml_dtypes.bfloat16)))
        dprow.append(np.ascontiguousarray(
            np.concatenate(dr)[None, :].astype(ml_dtypes.bfloat16)))

    binload = deg[arr].sum(axis=0)
    return dict(
        perm=perm, TL=TL, TH=TH, NPC=NPC, NTOT=NTOT, meta=meta,
        idxlow=idxlow, idxhigh=idxhigh, dpos=dpos, dprow=dprow,
        binload=binload,
    )


# ----------------------------------------------------------------------------
# Device program
# ----------------------------------------------------------------------------

def _build_program(NT, TL, TH, split, meta):
    import concourse.bacc as bacc
    import concourse.mybir as mybir
    import concourse.tile as tile

    dt = mybir.dt
    F32 = dt.float32
    BF = dt.bfloat16
    F8 = dt.float8e4
    U8 = dt.uint8
    I16 = dt.int16
    AF = mybir.ActivationFunctionType
    OP = mybir.AluOpType

    NPC = NT * P
    NTOT = NCORES * NPC
    TT = TL + TH
    assert NT % SUP == 0
    NSUP = NT // SUP
    NQ = int(os.environ.get("GNN_QUEUES") or 2)
    KA = min(int(os.environ.get("GNN_KAHEAD") or K_AHEAD), NSUP)
    USE_PREP = os.environ.get("GNN_PREP", "0") != "0"

    # per-super section geometry (identical across cores)
    LS = list(zip(meta["l0"], meta["l1"]))
    HS = list(zip(meta["h0"], meta["h1"]))
    NLO_S = [(a + b) * P for a, b in LS]      # low slots per super
    NHI_S = [(a + b) * P for a, b in HS]
    NST_S = [(NLO_S[S] + NHI_S[S]) // P for S in range(NSUP)]
    MAXLO = max(NLO_S) // P                   # tiles
    MAXHI = max(NHI_S) // P
    MAXNST = max(NST_S)

    def _cum(xs):
        out = [0]
        for x in xs:
            out.append(out[-1] + x)
        return out
    il_off = _cum([n // 16 for n in NLO_S])   # idx cols (int16)
    ih_off = _cum([n // 16 for n in NHI_S])
    dp_off_s = _cum(NST_S)                    # dpos cols
    dr_off_s = _cum([n * P for n in NST_S])   # dprow cols

    nc = bacc.Bacc("TRN2", target_bir_lowering=False, debug=False,
                   num_devices=NCORES,
                   dynamic_dma_scratch_size=int(os.environ.get("GNN_SCRATCH")
                                                or 24576),
                   num_swdge_queues=NQ)

    obst_p = nc.dram_tensor("obst", [OBS_D, NPC], BF, kind="ExternalInput")
    W1_p = nc.dram_tensor("w1", [OBS_D, HID], BF, kind="ExternalInput")
    b1_p = nc.dram_tensor("b1", [HID, 1], F32, kind="ExternalInput")
    W2_p = nc.dram_tensor("w2", [HID, H], BF, kind="ExternalInput")
    b2_p = nc.dram_tensor("b2", [H, 1], F32, kind="ExternalInput")
    Wg_p = [nc.dram_tensor(f"wg{i}", [H, H], BF, kind="ExternalInput")
            for i in (1, 2)]
    Wgal_p = [nc.dram_tensor(f"wgal{i}", [H, NH], BF, kind="ExternalInput")
              for i in (1, 2)]
    Wgar_p = [nc.dram_tensor(f"wgar{i}", [H, NH], BF, kind="ExternalInput")
              for i in (1, 2)]
    iota_p = nc.dram_tensor("iota", [P, P], BF, kind="ExternalInput")
    identf_p = nc.dram_tensor("identf", [P, P], F32, kind="ExternalInput")
    il_p = nc.dram_tensor("idxlow", [P, il_off[-1]], I16, kind="ExternalInput")
    ih_p = nc.dram_tensor("idxhigh", [P, ih_off[-1]], I16,
                          kind="ExternalInput")
    dprow_p = nc.dram_tensor("dprow", [1, dr_off_s[-1]], BF,
                             kind="ExternalInput")
    iotac_p = nc.dram_tensor("iotac", [P, 1], BF, kind="ExternalInput")
    dpos_p = nc.dram_tensor("dposslab", [P, dp_off_s[-1]], BF,
                            kind="ExternalInput")
    out_p = nc.dram_tensor("out", [NPC, 3 * H], F32, kind="ExternalOutput")

    tab_loc = [nc.dram_tensor(f"tab{i}_loc", [NPC, ROWB], F8) for i in (1, 2)]
    tab_full = [nc.dram_tensor(f"tab{i}_full", [NTOT, ROWB], F8,
                               addr_space="Shared") for i in (1, 2)]
    DBG = bool(os.environ.get("GNN_DBG"))
    if DBG:
        dbg_tab = nc.dram_tensor("dbg_tab", [NTOT, ROWB], F8,
                                 kind="ExternalOutput")
        dbg_gl = nc.dram_tensor("dbg_gl", [P, MAXLO * ROWB], F8,
                                kind="ExternalOutput")
        dbg_tabloc = nc.dram_tensor("dbg_tabloc", [NPC, ROWB], F8,
                                    kind="ExternalOutput")

    groups = [list(range(NCORES))]
    # One sem per DMASW lane: tile_sem_assignment rotates Pool-engine DMA
    # insts over 8 global DMASW lanes in issue order and emits consumer
    # waits as (lane sem) >= 16 * lane_tick; the descriptor-baked sem must
    # therefore be unique per lane, mirroring that rotation exactly.
    gsem = [nc.alloc_semaphore(f"gsem{i}") for i in range(8)]

    with tile.TileContext(nc) as tc:
        with (
            tc.tile_pool(name="const", bufs=1) as constp,
            tc.tile_pool(name="obst", bufs=2) as obstp,
            tc.tile_pool(name="enc", bufs=2) as encp,
            tc.tile_pool(name="rows", bufs=3) as rowsp,
            tc.tile_pool(name="gath", bufs=KA + 1) as gathp,
            tc.tile_pool(name="small", bufs=3) as smallp,
            tc.tile_pool(name="rhs", bufs=2) as rhsp,
            tc.tile_pool(name="bt", bufs=2) as btp,
            tc.tile_pool(name="pe", bufs=2, space="PSUM") as pep,
            tc.tile_pool(name="pacc", bufs=2, space="PSUM") as paccp,
            tc.tile_pool(name="ptr", bufs=1, space="PSUM") as ptrp,
            tc.tile_pool(name="prod", bufs=1, space="PSUM") as prodp,
            tc.tile_pool(name="pers", bufs=2, space="PSUM") as persp,
        ):
            # ---------------- prologue ----------------
            iota_sb = constp.tile([P, P], BF, tag="iota")
            nc.sync.dma_start(iota_sb[:], iota_p[:, :])
            ident = constp.tile([P, P], F32, tag="ident")
            nc.sync.dma_start(ident[:], identf_p[:, :])
            iotac_sb = constp.tile([P, 1], BF, tag="iotac")
            nc.sync.dma_start(iotac_sb[:], iotac_p[:, :])
            il_sb = constp.tile([P, il_off[-1]], I16, tag="il")
            nc.scalar.dma_start(il_sb[:], il_p[:, :])
            ih_sb = constp.tile([P, ih_off[-1]], I16, tag="ih")
            nc.scalar.dma_start(ih_sb[:], ih_p[:, :])
            dpos_sb = constp.tile([P, dp_off_s[-1]], BF, tag="dpos")
            nc.scalar.dma_start(dpos_sb[:], dpos_p[:, :])
            er_sb0 = constp.tile([P, NT * NH], BF, tag="er_sb0")
            er_sb1 = constp.tile([P, NT * NH], BF, tag="er_sb1")
            er_sb = [er_sb0, er_sb1]

            W1_sb = []
            for k in range(2):
                t = constp.tile([P, HID], BF, tag=f"w1_{k}")
                nc.sync.dma_start(t[:], W1_p[k * P:(k + 1) * P, :])
                W1_sb.append(t)
            W2_sb = []
            for m in range(4):
                t = constp.tile([P, H], BF, tag=f"w2_{m}")
                nc.sync.dma_start(t[:], W2_p[m * P:(m + 1) * P, :])
                W2_sb.append(t)
            b1_sb = []
            for m in range(4):
                t = constp.tile([P, 1], F32, tag=f"b1_{m}")
                nc.sync.dma_start(t[:], b1_p[m * P:(m + 1) * P, :])
                b1_sb.append(t)
            b2_sb = constp.tile([P, 1], F32, tag="b2")
            nc.sync.dma_start(b2_sb[:], b2_p[:, :])
            Wg_sb, Wgal_sb, Wgar_sb = [], [], []
            for i in range(2):
                t = constp.tile([P, H], BF, tag=f"wg_{i}")
                nc.sync.dma_start(t[:], Wg_p[i][:, :])
                Wg_sb.append(t)
                t = constp.tile([P, NH], BF, tag=f"wgal_{i}")
                nc.sync.dma_start(t[:], Wgal_p[i][:, :])
                Wgal_sb.append(t)
                t = constp.tile([P, NH], BF, tag=f"wgar_{i}")
                nc.sync.dma_start(t[:], Wgar_p[i][:, :])
                Wgar_sb.append(t)

            # ------------- chunked AllGather -------------
            # The collective instruction is a dispatch-only trigger (ncfw
            # moves the data), so the first half of each table AllGather can
            # be issued as soon as its rows are written — overlapping the
            # encoder (AG1) / layer-1 (AG2) tails; only the second half sits
            # on the critical path.
            NCH = 2 if NT % 4 == 0 else 1

            def ag_chunk(li, k):
                rows = NPC // NCH
                gr = NTOT // NCH
                nc.gpsimd.collective_compute(
                    "AllGather", OP.bypass, replica_groups=groups,
                    ins=[tab_loc[li][k * rows:(k + 1) * rows, :]],
                    outs=[tab_full[li][k * gr:(k + 1) * gr, :]])

            # ------------- gather prep machinery -------------
            gtiles = {}
            prep_ctr = [0]   # mirrors tile's DMASW lane rotation

            def issue_prep(li, S):
                tabf = tab_full[li]
                pkw0 = (dict(prepare_only=True,
                             sem=gsem[prep_ctr[0] % 8])
                        if USE_PREP else {})
                pkw1 = (dict(prepare_only=True,
                             sem=gsem[(prep_ctr[0] + 1) % 8])
                        if USE_PREP else {})
                prep_ctr[0] += 2
                cl, ch = NLO_S[S] // P, NHI_S[S] // P
                gl = gathp.tile([P, MAXLO * ROWB], F8, tag="glow")
                nc.gpsimd.dma_gather(
                    out_ap=gl[:, 0:cl * ROWB]
                        .rearrange("p (c e) -> p c e", e=ROWB),
                    in_ap=tabf[:, :],
                    idxs_ap=il_sb[:, il_off[S]:il_off[S + 1]],
                    num_idxs=NLO_S[S], num_idxs_reg=meta["nrl"][S],
                    elem_size=ROWB,
                    single_packet=False, queue_num=0, **pkw0)
                gh = gathp.tile([P, MAXHI * ROWB], F8, tag="ghigh")
                nc.gpsimd.dma_gather(
                    out_ap=gh[:, 0:ch * ROWB]
                        .rearrange("p (c e) -> p c e", e=ROWB),
                    in_ap=tabf[split:NTOT, :],
                    idxs_ap=ih_sb[:, ih_off[S]:ih_off[S + 1]],
                    num_idxs=NHI_S[S], num_idxs_reg=meta["nrh"][S],
                    elem_size=ROWB,
                    single_packet=False, queue_num=1 % NQ, **pkw1)
                gtiles[(li, S)] = (gl, gh)

            def fire_all():
                """Fire every untriggered prep (only current-layer preps may
                be pending when this is called)."""
                if not USE_PREP:
                    return
                for q in range(min(NQ, 2)):
                    nc.gpsimd.trigger_dma(count=None, queue_num=q)

            def table_products(zTb_chunk, row0, li):
                pr = prodp.tile([P, H + 2 * NH], F32, tag="pr")
                nc.tensor.matmul(pr[:, 0:H], lhsT=zTb_chunk, rhs=Wg_sb[li][:],
                                 start=True, stop=True)
                nc.tensor.matmul(pr[:, H:H + NH], lhsT=zTb_chunk,
                                 rhs=Wgal_sb[li][:], start=True, stop=True)
                nc.tensor.matmul(pr[:, H + NH:H + 2 * NH], lhsT=zTb_chunk,
                                 rhs=Wgar_sb[li][:], start=True, stop=True)
                rowt = rowsp.tile([P, H + NH], F8, tag="rowt")
                nc.vector.tensor_copy(rowt[:, 0:H], pr[:, 0:H])
                nc.vector.tensor_copy(rowt[:, H:H + NH], pr[:, H:H + NH])
                nc.sync.dma_start(
                    tab_loc[li][row0:row0 + P, 0:H + NH], rowt[:])
                D = row0 // P
                nc.vector.tensor_copy(er_sb[li][:, D * NH:(D + 1) * NH],
                                      pr[:, H + NH:H + 2 * NH])

            # Gather tiles are partially skipped (trailing -1 indices), so
            # zero them once: stale SBUF could hold fp8 NaN patterns that
            # would poison 0*NaN in the scatter matmul.
            for _ in range(KA + 1):
                t = gathp.tile([P, MAXLO * ROWB], F8, tag="glow")
                nc.gpsimd.memset(t[:], 0.0)
                t = gathp.tile([P, MAXHI * ROWB], F8, tag="ghigh")
                nc.gpsimd.memset(t[:], 0.0)

            # ----- layer-1 gather desc prep (hides under encoder+AG) -----
            if USE_PREP:
                for S in range(KA):
                    issue_prep(0, S)

            # ---------------- phase E: encoder ----------------
            for pt in range(NT // 2):
                n0 = pt * 2 * P
                obsT = []
                for k in range(2):
                    t = obstp.tile([P, 2 * P], BF, tag="obsT")
                    nc.sync.dma_start(t[:], obst_p[k * P:(k + 1) * P,
                                                   n0:n0 + 2 * P])
                    obsT.append(t)
                hT = []
                for m in range(4):
                    ph = pep.tile([P, 2 * P], F32, tag="pe")
                    for k in range(2):
                        nc.tensor.matmul(
                            ph[:], lhsT=W1_sb[k][:, m * P:(m + 1) * P],
                            rhs=obsT[k][:], start=(k == 0), stop=(k == 1))
                    h = encp.tile([P, 2 * P], BF, tag=f"h{m}")
                    nc.vector.tensor_scalar(
                        out=h[:], in0=ph[:], scalar1=b1_sb[m][:, 0:1],
                        scalar2=0.0, op0=OP.add, op1=OP.max)
                    hT.append(h)
                pz = pep.tile([P, 2 * P], F32, tag="pe")
                for m in range(4):
                    nc.tensor.matmul(pz[:], lhsT=W2_sb[m][:], rhs=hT[m][:],
                                     start=(m == 0), stop=(m == 3))
                z1T = encp.tile([P, 2 * P], F32, tag="z1T")
                nc.vector.tensor_scalar(
                    out=z1T[:], in0=pz[:], scalar1=b2_sb[:, 0:1],
                    scalar2=0.0, op0=OP.add, op1=OP.max)
                z1Tb = encp.tile([P, 2 * P], BF, tag="z1Tb")
                nc.vector.tensor_copy(z1Tb[:], z1T[:])
                for k in range(2):
                    ptr = ptrp.tile([P, P], F32, tag="ptr")
                    nc.tensor.transpose(ptr[:], z1T[:, k * P:(k + 1) * P],
                                        ident[:])
                    zrow = rowsp.tile([P, P], F32, tag="zrows")
                    nc.vector.tensor_copy(zrow[:], ptr[:])
                    nc.scalar.dma_start(
                        out_p[n0 + k * P:n0 + (k + 1) * P, 0:H], zrow[:])
                    table_products(z1Tb[:, k * P:(k + 1) * P], n0 + k * P, 0)
                if NCH == 2 and pt == NT // 4 - 1:
                    ag_chunk(0, 0)

            ag_chunk(0, NCH - 1)

            # ---------------- edge pass ----------------
            def epilogue(D, acc, out_col, build_next):
                zp = smallp.tile([P, NH], F32, tag="zp")
                nc.vector.tensor_scalar_add(zp[:], acc[:, H:TAB], 1e-9)
                zrec = smallp.tile([P, NH], F32, tag="zrec")
                nc.vector.reciprocal(zrec[:], zp[:])
                zo = rowsp.tile([P, H], F32, tag="zo")
                nc.vector.scalar_tensor_tensor(
                    out=zo[:].rearrange("p (h d) -> p h d", h=NH),
                    in0=acc[:, 0:H].rearrange("p (h d) -> p h d", h=NH),
                    scalar=0.0,
                    in1=zrec[:].unsqueeze(2).to_broadcast([P, NH, HD]),
                    op0=OP.max, op1=OP.mult)
                nc.scalar.dma_start(
                    out_p[D * P:(D + 1) * P, out_col:out_col + H], zo[:])
                if build_next:
                    pzt = ptrp.tile([P, P], F32, tag="ptr")
                    nc.tensor.transpose(pzt[:], zo[:], ident[:])
                    zTb = rowsp.tile([P, P], BF, tag="zTb")
                    nc.vector.tensor_copy(zTb[:], pzt[:])
                    table_products(zTb[:], D * P, 1)

            def edge_block(g, er_ps, er_off, dp_off, accs,
                           S, binmap, out_col, build_next, is_high):
                """One low/high block of a super: batched attn + per-tile mm."""
                n = len(binmap)
                g3 = g[:, 0:n * ROWB].rearrange("p (c e) -> p c e", e=ROWB)
                gfeat = g3[:, :, 0:H]
                gel = g3[:, :, H:H + NH]
                NMX = MAXHI if is_high else MAXLO
                e_t = smallp.tile([P, NMX * NH], F32,
                                  tag="e_th" if is_high else "e_tl")
                nc.vector.tensor_add(
                    e_t[:, 0:n * NH].rearrange("p (c e) -> p c e", e=NH),
                    gel,
                    er_ps[:, er_off * NH:(er_off + n) * NH]
                        .rearrange("p (c e) -> p c e", e=NH))
                lr = smallp.tile([P, NMX * NH], F32,
                                 tag="lrh" if is_high else "lrl")
                nc.vector.scalar_tensor_tensor(
                    out=lr[:, 0:n * NH], in0=e_t[:, 0:n * NH], scalar=0.2,
                    in1=e_t[:, 0:n * NH],
                    op0=OP.mult, op1=OP.max)
                rhs = rhsp.tile([P, NMX * TAB], BF,
                                tag="rhsh" if is_high else "rhsl")
                r3 = rhs[:, 0:n * TAB].rearrange("p (c e) -> p c e", e=TAB)
                nc.scalar.activation(
                    r3[:, :, H:TAB],
                    lr[:, 0:n * NH].rearrange("p (c e) -> p c e", e=NH),
                    AF.Exp)
                bt = btp.tile([P, NMX * P], BF,
                              tag="bth" if is_high else "btl")
                nc.vector.tensor_tensor(
                    out=bt[:, 0:n * P].rearrange("p (c e) -> p c e", e=P),
                    in0=dpos_sb[:, dp_off:dp_off + n].unsqueeze(2)
                        .to_broadcast([P, n, P]),
                    in1=iota_sb[:].unsqueeze(1).to_broadcast([P, n, P]),
                    op=OP.is_equal)
                nc.vector.tensor_tensor(
                    out=r3[:, :, 0:H].rearrange("p c (h d) -> p c h d", h=NH),
                    in0=gfeat.rearrange("p c (h d) -> p c h d", h=NH),
                    in1=r3[:, :, H:TAB].unsqueeze(3)
                        .to_broadcast([P, n, NH, HD]),
                    op=OP.mult)
                for c in range(n):
                    b = binmap[c]
                    first = c == 0 or binmap[c - 1] != b
                    last = c == n - 1 or binmap[c + 1] != b
                    D = S * SUP + b
                    if not is_high and first:
                        acc_new = paccp.tile([P, TAB], F32, tag="acc")
                        accs[b] = acc_new
                    nc.tensor.matmul(
                        accs[b][:], lhsT=bt[:, c * P:(c + 1) * P],
                        rhs=r3[:, c, :],
                        start=(not is_high and first),
                        stop=(is_high and last))
                    if is_high and last:
                        epilogue(D, accs[b], out_col, build_next)

            def edge_pass(li, out_col, build_next):
                accs = [None] * SUP
                # Fence: the first trigger must not fire before the AllGather
                # has written tab_full[li].  An SP probe read of the table
                # picks up the collective-completion wait; a Pool-engine copy
                # of the probe then blocks the GpSimd engine (and hence the
                # triggers behind it) until that data has landed.
                probe = smallp.tile([P, 2], F8, tag="probe")
                nc.sync.dma_start(probe[:], tab_full[li][0:P, 0:2])
                fence = smallp.tile([P, 2], F8, tag="fence")
                nc.gpsimd.tensor_copy(fence[:], probe[:])
                fire_all()
                for S in range(NSUP):
                    nst = NST_S[S]
                    lmap = ([0] * LS[S][0]) + ([1] * LS[S][1])
                    hmap = ([0] * HS[S][0]) + ([1] * HS[S][1])
                    dprow = btp.tile([P, MAXNST * P], BF, tag="dprow")
                    nc.sync.dma_start(
                        dprow[:, 0:nst * P],
                        dprow_p[0:1, dr_off_s[S]:dr_off_s[S + 1]]
                        .to_broadcast([P, nst * P]))
                    ball = btp.tile([P, MAXNST * P], BF, tag="ball")
                    nc.vector.tensor_tensor(
                        out=ball[:, 0:nst * P]
                            .rearrange("p (c e) -> p c e", e=P),
                        in0=iotac_sb[:, 0:1].unsqueeze(2)
                            .to_broadcast([P, nst, P]),
                        in1=dprow[:, 0:nst * P]
                            .rearrange("p (c e) -> p c e", e=P),
                        op=OP.is_equal)
                    er_ps = persp.tile([P, MAXNST * NH], F32, tag="er_ps")
                    for c, b in enumerate(lmap + hmap):
                        D = S * SUP + b
                        nc.tensor.matmul(
                            er_ps[:, c * NH:(c + 1) * NH],
                            lhsT=ball[:, c * P:(c + 1) * P],
                            rhs=er_sb[li][:, D * NH:(D + 1) * NH],
                            start=True, stop=True)

                    if not USE_PREP:
                        issue_prep(li, S)
                    gl, gh = gtiles.pop((li, S))
                    if DBG and li == 1 and S == 0:
                        nc.sync.dma_start(dbg_gl[:, :], gl[:])
                        nc.sync.dma_start(dbg_tab[:, :], tab_full[1][:, :])
                    edge_block(gl[:], er_ps, 0, dp_off_s[S], accs,
                               S, lmap, out_col, build_next, False)
                    edge_block(gh[:], er_ps, len(lmap),
                               dp_off_s[S] + len(lmap),
                               accs, S, hmap, out_col, build_next, True)

                    # prep K_AHEAD supers forward (flows into next layer);
                    # current-layer preps are fired immediately, next-layer
                    # preps stay untriggered until the next edge_pass's
                    # post-AllGather fire_all.
                    if USE_PREP:
                        nS = S + KA
                        if nS < NSUP:
                            issue_prep(li, nS)
                            fire_all()
                        elif li == 0 and nS - NSUP < min(KA, NSUP):
                            issue_prep(1, nS - NSUP)
                    if (NCH == 2 and li == 0 and build_next
                            and S == NSUP // 2 - 1):
                        ag_chunk(1, 0)

            edge_pass(0, H, True)

            if DBG:
                nc.sync.dma_start(dbg_tabloc[:, :], tab_loc[1][:, :])
            ag_chunk(1, NCH - 1)

            edge_pass(1, 2 * H, False)

    if USE_PREP:
        # Tile emits data-consumer waits on its own per-lane DMASW{k} sems
        # (>= 16 * lane tick) but leaves the caller's sem= in the prep's
        # OnUpdate[0] — which is what the DMA descriptors actually bump.
        # Point OnUpdate[0] at the lane sem so the waits mean something.
        def _walk():
            for f in nc.m.functions:
                for bb in f.blocks:
                    for i in bb.instructions:
                        yield i
        lane_sem_id = {}
        for i in _walk():
            si = i.sync_info
            if not si:
                continue
            for x in list(si.on_wait or []) + list(si.on_update or []):
                nm = x.ant_name
                if nm.startswith("DMASW"):
                    lane_sem_id[int(nm[5:].split("_")[0])] = x.id
        fixed = 0
        for i in _walk():
            if (type(i).__name__ == "InstDMAGatherAnt"
                    and getattr(i, "gen_mode", 0) == 1):
                lane = i.bass_scheduled_proc - 11
                if lane in lane_sem_id:
                    i.sync_info.on_update[0].id = lane_sem_id[lane]
                    fixed += 1
        assert fixed == 0 or len(lane_sem_id) > 0

    nc.compile()
    return nc


# ----------------------------------------------------------------------------
# Driver
# ----------------------------------------------------------------------------

def _make_blockdiag(a):
    bd = np.zeros((H, NH), np.float32)
    for h in range(NH):
        bd[h * HD:(h + 1) * HD, h] = a[h]
    return bd


def run_gnn(inputs, n_tiles_per_core=52, trace=False):
    import ml_dtypes
    bf16 = ml_dtypes.bfloat16

    t_start = time.time()
    obs = np.asarray(inputs["obs"], np.float32)
    src = np.asarray(inputs["src"], np.int64)
    dst = np.asarray(inputs["dst"], np.int64)
    N = obs.shape[0]

    NTOT_ = NCORES * n_tiles_per_core * P
    split = min(SPLIT, NTOT_ // 2)
    prep = _host_prepare(src, dst, n_tiles_per_core, split)
    NT = n_tiles_per_core
    TL, TH, NPC, NTOT = prep["TL"], prep["TH"], prep["NPC"], prep["NTOT"]
    perm = prep["perm"]

    al1bd = _make_blockdiag(np.asarray(inputs["al1"], np.float32))
    ar1bd = _make_blockdiag(np.asarray(inputs["ar1"], np.float32))
    al2bd = _make_blockdiag(np.asarray(inputs["al2"], np.float32))
    ar2bd = _make_blockdiag(np.asarray(inputs["ar2"], np.float32))
    Wg1 = np.asarray(inputs["Wg1"], np.float32)
    Wg2 = np.asarray(inputs["Wg2"], np.float32)
    shared = {
        "w1": np.asarray(inputs["W1"], np.float32).astype(bf16),
        "b1": np.asarray(inputs["b1"], np.float32).reshape(HID, 1),
        "w2": np.asarray(inputs["W2"], np.float32).astype(bf16),
        "b2": np.asarray(inputs["b2"], np.float32).reshape(H, 1),
        "wg1": Wg1.astype(bf16), "wg2": Wg2.astype(bf16),
        "wgal1": (Wg1 @ al1bd).astype(bf16),
        "wgar1": (Wg1 @ ar1bd).astype(bf16),
        "wgal2": (Wg2 @ al2bd).astype(bf16),
        "wgar2": (Wg2 @ ar2bd).astype(bf16),
        "iota": np.tile(np.arange(P, dtype=np.float32)[None, :],
                        (P, 1)).astype(bf16),
        "identf": np.eye(P, dtype=np.float32),
        "iotac": np.arange(P, dtype=np.float32).reshape(P, 1).astype(bf16),
    }

    obs_pad = np.zeros((NTOT, OBS_D), np.float32)
    obs_pad[:N] = obs
    obs_perm = obs_pad[perm]

    in_maps = []
    for c in range(NCORES):
        m = dict(shared)
        m["obst"] = np.ascontiguousarray(
            obs_perm[c * NPC:(c + 1) * NPC].T).astype(bf16)
        m["idxlow"] = prep["idxlow"][c]
        m["idxhigh"] = prep["idxhigh"][c]
        m["dprow"] = prep["dprow"][c]
        m["dposslab"] = prep["dpos"][c]
        in_maps.append(m)

    t_prep = time.time()
    nc = _build_program(NT, TL, TH, split, prep["meta"])
    t_build = time.time()

    from concourse.bass_utils import run_bass_kernel_spmd
    if trace:
        _ensure_ntff_hook()
    res = run_bass_kernel_spmd(nc, in_maps, core_ids=list(range(NCORES)),
                               trace=trace)
    t_run = time.time()

    full = np.concatenate([res.results[c]["out"] for c in range(NCORES)],
                          axis=0)
    out = np.empty((N, 3 * H), np.float32)
    keep = perm < N
    out[perm[keep]] = full[keep]

    LAST_INFO.clear()
    if os.environ.get("GNN_DBG"):
        LAST_INFO["dbg"] = {k: [res.results[c].get(k) for c in range(NCORES)]
                            for k in ("dbg_tab", "dbg_gl", "dbg_tabloc")}
        LAST_INFO["prep"] = prep
    LAST_INFO.update(dict(
        exec_time_ns=res.exec_time_ns, TL=TL, TH=TH,
        binload_max=int(prep["binload"].max()),
        t_prep=t_prep - t_start, t_build=t_build - t_prep,
        t_run=t_run - t_build,
        profile_json=getattr(res, "profile_json", None),
    ))
    return out


def kernel(**inputs):
    return run_gnn(inputs, n_tiles_per_core=52,
                   trace=bool(os.environ.get("GNN_TRACE")))


# revision 41
# speedup vs baseline: 1.1783x; 1.1783x over previous
"""Distributed multi-head GAT (encoder + 2 GAT layers) on 8 TRN2 NeuronCores.

Strategy (graph/data parallel, dst-ownership sharding):
  * Nodes are permuted and dealt into 8*NT bins of 128 so each bin (= one
    PSUM dst-tile) has a near-equal number of incoming edges.  Edges live
    with the core that owns their dst node.

  * Each core encodes its shard (obs -> z1, bf16 MLP) and builds a packed
    256B fp8 table row per node: feat[0:128] | el[128:136]; rows are
    AllGathered so any core can gather src rows.
  * Edge pass: per super-tile (SUP bins), batched dma_gather pulls the
    256B rows of that bin-group's edge srcs (low/high table halves on two
    SWDGE queues so int16 indices can address them).  Section sizes are
    baked per (super, bin, half) as the max over cores, and each gather's
    trailing pads are -1 indices the ucode skips (num_idxs_reg = the
    cross-core-uniform real count), which trims descriptor generation on
    GpSimd — the dominant cost — by ~13%.
  * Attention: e = el[src]+er[dst] (er via one-hot matmul), then
    ex = exp(max(e, 0.2e)) (exact leaky-relu+exp, softmax max-subtraction
    skipped: inputs are O(0.1) so exp is safe; matches up to the 1e-9
    epsilon).  Messages feat*ex reduce into the bin's PSUM accumulator via
    a one-hot matmul; the same matmul accumulates the softmax denominator.
  * Epilogue per bin: out = relu(acc_feat)/(acc_z+1e-9) per head; also
    emits the next layer's packed table row + er entries.
"""

import os
import sys
import time

import numpy as np

for _p in ("/opt/trn_rl_repo", "/root/.axon_site/_ro/trn_rl_repo"):
    if os.path.isdir(_p) and _p not in sys.path:
        sys.path.insert(0, _p)

P = 128
NCORES = 8
OBS_D = 256
HID = 512
H = 128          # h_dim
NH = 8           # heads
HD = 16          # head dim
ROWB = 256       # table row (fp8 elems): feat[0:128] | el[128:136] | pad
TAB = H + NH     # rhs row: msg(128) | ex(8)
SUP = 2          # bins per gather super-tile
SPLIT = 32768    # low/high table split for int16 gather indices
PAD_SENTINEL = 200.0
# Supers of gather-descriptor prep-ahead.  Capped at 3: Tile rotates SWDGE
# preps over 8 global DMASW sem lanes (2 preps/super), and a lane's >=16*tick
# wait is only exact when at most one fired gather is outstanding per lane —
# fire-ahead of 4 supers (8 preps, one per lane) is the provable bound, and
# the gather pool's WAR edge (bufs=KA+1) keeps same-lane preps serialized.
K_AHEAD = 3

LAST_INFO = {}


def _ensure_ntff_hook():
    """Register the axon NTFF profile hook if the image's antenv lacks it."""
    try:
        import types

        import antenv
        try:
            from antenv import axon_hooks  # noqa: F401
            return
        except ImportError:
            pass
        m = types.ModuleType("antenv.axon_hooks")
        _h = [None]
        m.set_axon_ntff_profile_hook = lambda hook: _h.__setitem__(0, hook)
        m.get_axon_ntff_profile_hook = lambda: _h[0]
        sys.modules["antenv.axon_hooks"] = m
        antenv.axon_hooks = m
        from trn_agent_boot.trn_boot import _ntff_profile_via_ctypes
        m.set_axon_ntff_profile_hook(
            _ntff_profile_via_ctypes("/opt/axon/libaxon_pjrt.so"))
    except Exception as e:  # profiling is best-effort
        print(f"ntff hook setup failed: {e}")


# ----------------------------------------------------------------------------
# Host-side preprocessing
# ----------------------------------------------------------------------------

def _wrap16(a):
    """[n] -> [128, n/16] int16 in the dma_gather wrapped layout:
    index j lives at partition j%16, col j//16, replicated to all 8 groups."""
    n = a.shape[0]
    w = a.reshape(n // 16, 16).T.astype(np.int16)    # [16, n/16]
    return np.ascontiguousarray(np.tile(w, (8, 1)))


def _host_prepare(src, dst, n_tiles_per_core, split):
    """Balance bins, split edges by src table half, build gather slabs."""
    import ml_dtypes

    NT = n_tiles_per_core
    NPC = NT * P
    NTOT = NCORES * NPC
    NBINS = NCORES * NT
    E = src.shape[0]

    deg = np.bincount(dst, minlength=NTOT).astype(np.int64)
    order = np.argsort(-deg, kind="stable")
    arr = order.reshape(P, NBINS).copy()
    arr[1::2] = arr[1::2, ::-1]
    perm = arr.T.reshape(-1)                       # g -> orig node
    pos = np.empty(NTOT, np.int64)
    pos[perm] = np.arange(NTOT)                    # orig node -> g

    srcg = pos[src]
    dstg = pos[dst]
    # Table rows are laid out chunk-major so each AllGather chunk (the
    # first/second half of every core's shard) lands in one contiguous
    # slice of tab_full; remap gather indices accordingly.
    nch = 1
    if nch == 2:
        half = NPC // 2
        sc, sr = srcg // NPC, srcg % NPC
        k = sr // half
        srcg = k * (NTOT // 2) + sc * half + (sr - k * half)
    binid = dstg // P
    low = srcg < split

    nlo = np.bincount(binid[low], minlength=NBINS)
    nhi = np.bincount(binid[~low], minlength=NBINS)
    TL = max(1, int(np.ceil(nlo.max() / P)))
    TH = max(1, int(np.ceil(nhi.max() / P)))
    TT = TL + TH

    gkey = binid * 2 + (~low).astype(np.int64)
    eorder = np.argsort(gkey, kind="stable")
    counts = np.bincount(gkey, minlength=2 * NBINS)
    starts = np.concatenate([[0], np.cumsum(counts)[:-1]])
    rank = np.arange(E) - starts[gkey[eorder]]
    ghigh = gkey[eorder] % 2
    slot = (gkey[eorder] // 2) * (TT * P) + ghigh * (TL * P) + rank

    ES = NBINS * TT * P
    sg = np.zeros(ES, np.int64)
    isreal = np.zeros(ES, bool)
    dposf = np.full(ES, PAD_SENTINEL, np.float32)
    sg[slot] = srcg[eorder]
    isreal[slot] = True
    dposf[slot] = (dstg[eorder] % P).astype(np.float32)

    # high-section pads must index within the high table half
    sg[~isreal & (np.arange(ES) % (TT * P) >= TL * P)] = split

    sg = sg.reshape(NCORES, NT, TT * P)
    dposf = dposf.reshape(NCORES, NT, TT * P).astype(ml_dtypes.bfloat16)

    # Per-(local bin, half) tile counts: max over cores, so the section
    # layout (baked into the SPMD program) fits every core.  Within a
    # super the second bin's tail pads become -1 (skipped by the gather;
    # num_idxs_reg = the identical-across-cores real+0-pad count).
    nlo2 = nlo.reshape(NCORES, NT)
    nhi2 = nhi.reshape(NCORES, NT)
    TLOb = np.maximum(1, -(-nlo2.max(axis=0) // P))      # [NT]
    THIb = np.maximum(1, -(-nhi2.max(axis=0) // P))

    NSUP = NT // SUP
    meta = dict(l0=[], l1=[], h0=[], h1=[], nrl=[], nrh=[])
    for S in range(NSUP):
        b0, b1 = SUP * S, SUP * S + 1
        l0, l1 = int(TLOb[b0]), int(TLOb[b1])
        h0, h1 = int(THIb[b0]), int(THIb[b1])
        n1l = max(1, int(nlo2[:, b1].max()))
        n1h = max(1, int(nhi2[:, b1].max()))
        meta["l0"].append(l0)
        meta["l1"].append(l1)
        meta["h0"].append(h0)
        meta["h1"].append(h1)
        meta["nrl"].append(l0 * P + n1l)
        meta["nrh"].append(h0 * P + n1h)

    idxlow, idxhigh, dpos, dprow = [], [], [], []
    for c in range(NCORES):
        il, ih, dp, dr = [], [], [], []
        for S in range(NSUP):
            b0, b1 = SUP * S, SUP * S + 1
            l0, l1 = meta["l0"][S], meta["l1"][S]
            h0, h1 = meta["h0"][S], meta["h1"][S]
            lo = np.concatenate([sg[c, b0, :l0 * P], sg[c, b1, :l1 * P]])
            lo[meta["nrl"][S]:] = -1
            hi = np.concatenate([sg[c, b0, TL * P:TL * P + h0 * P],
                                 sg[c, b1, TL * P:TL * P + h1 * P]]) - split
            hi[meta["nrh"][S]:] = -1
            il.append(_wrap16(lo))
            ih.append(_wrap16(hi))
            dpl = np.concatenate(
                [dposf[c, b0, :l0 * P], dposf[c, b1, :l1 * P]]
            ).reshape(l0 + l1, P)
            dph = np.concatenate(
                [dposf[c, b0, TL * P:TL * P + h0 * P],
                 dposf[c, b1, TL * P:TL * P + h1 * P]]
            ).reshape(h0 + h1, P)
            both = np.concatenate([dpl, dph], 0)         # [nst_S, 128]
            dp.append(both.T)                            # [128, nst_S]
            dr.append(both.reshape(-1))                  # slot-order flat
        idxlow.append(np.ascontiguousarray(np.concatenate(il, 1)))
        idxhigh.append(np.ascontiguousarray(np.concatenate(ih, 1)))
        dpos.append(np.ascontiguousarray(
            np.concatenate(dp, 1).astype(ml_dtypes.bfloat16)))
        dprow.append(np.ascontiguousarray(
            np.concatenate(dr)[None, :].astype(ml_dtypes.bfloat16)))

    binload = deg[arr].sum(axis=0)
    return dict(
        perm=perm, TL=TL, TH=TH, NPC=NPC, NTOT=NTOT, meta=meta,
        idxlow=idxlow, idxhigh=idxhigh, dpos=dpos, dprow=dprow,
        binload=binload,
    )


# ----------------------------------------------------------------------------
# Device program
# ----------------------------------------------------------------------------

def _build_program(NT, TL, TH, split, meta):
    import concourse.bacc as bacc
    import concourse.mybir as mybir
    import concourse.tile as tile

    dt = mybir.dt
    F32 = dt.float32
    BF = dt.bfloat16
    F8 = dt.float8e4
    U8 = dt.uint8
    I16 = dt.int16
    AF = mybir.ActivationFunctionType
    OP = mybir.AluOpType

    NPC = NT * P
    NTOT = NCORES * NPC
    TT = TL + TH
    assert NT % SUP == 0
    NSUP = NT // SUP
    NQ = int(os.environ.get("GNN_QUEUES") or 2)
    KA = min(int(os.environ.get("GNN_KAHEAD") or K_AHEAD), NSUP)
    USE_PREP = os.environ.get("GNN_PREP", "0") != "0"

    # per-super section geometry (identical across cores)
    LS = list(zip(meta["l0"], meta["l1"]))
    HS = list(zip(meta["h0"], meta["h1"]))
    NLO_S = [(a + b) * P for a, b in LS]      # low slots per super
    NHI_S = [(a + b) * P for a, b in HS]
    NST_S = [(NLO_S[S] + NHI_S[S]) // P for S in range(NSUP)]
    MAXLO = max(NLO_S) // P                   # tiles
    MAXHI = max(NHI_S) // P
    MAXNST = max(NST_S)

    def _cum(xs):
        out = [0]
        for x in xs:
            out.append(out[-1] + x)
        return out
    il_off = _cum([n // 16 for n in NLO_S])   # idx cols (int16)
    ih_off = _cum([n // 16 for n in NHI_S])
    dp_off_s = _cum(NST_S)                    # dpos cols
    dr_off_s = _cum([n * P for n in NST_S])   # dprow cols

    nc = bacc.Bacc("TRN2", target_bir_lowering=False, debug=False,
                   num_devices=NCORES,
                   dynamic_dma_scratch_size=int(os.environ.get("GNN_SCRATCH")
                                                or 24576),
                   num_swdge_queues=NQ)

    obst_p = nc.dram_tensor("obst", [OBS_D, NPC], BF, kind="ExternalInput")
    W1_p = nc.dram_tensor("w1", [OBS_D, HID], BF, kind="ExternalInput")
    b1_p = nc.dram_tensor("b1", [HID, 1], F32, kind="ExternalInput")
    W2_p = nc.dram_tensor("w2", [HID, H], BF, kind="ExternalInput")
    b2_p = nc.dram_tensor("b2", [H, 1], F32, kind="ExternalInput")
    Wg_p = [nc.dram_tensor(f"wg{i}", [H, H], BF, kind="ExternalInput")
            for i in (1, 2)]
    Wgal_p = [nc.dram_tensor(f"wgal{i}", [H, NH], BF, kind="ExternalInput")
              for i in (1, 2)]
    Wgar_p = [nc.dram_tensor(f"wgar{i}", [H, NH], BF, kind="ExternalInput")
              for i in (1, 2)]
    iota_p = nc.dram_tensor("iota", [P, P], BF, kind="ExternalInput")
    identf_p = nc.dram_tensor("identf", [P, P], F32, kind="ExternalInput")
    il_p = nc.dram_tensor("idxlow", [P, il_off[-1]], I16, kind="ExternalInput")
    ih_p = nc.dram_tensor("idxhigh", [P, ih_off[-1]], I16,
                          kind="ExternalInput")
    dprow_p = nc.dram_tensor("dprow", [1, dr_off_s[-1]], BF,
                             kind="ExternalInput")
    iotac_p = nc.dram_tensor("iotac", [P, 1], BF, kind="ExternalInput")
    dpos_p = nc.dram_tensor("dposslab", [P, dp_off_s[-1]], BF,
                            kind="ExternalInput")
    out_p = nc.dram_tensor("out", [NPC, 3 * H], F32, kind="ExternalOutput")

    tab_loc = [nc.dram_tensor(f"tab{i}_loc", [NPC, ROWB], F8) for i in (1, 2)]
    tab_full = [nc.dram_tensor(f"tab{i}_full", [NTOT, ROWB], F8,
                               addr_space="Shared") for i in (1, 2)]
    DBG = bool(os.environ.get("GNN_DBG"))
    if DBG:
        dbg_tab = nc.dram_tensor("dbg_tab", [NTOT, ROWB], F8,
                                 kind="ExternalOutput")
        dbg_gl = nc.dram_tensor("dbg_gl", [P, MAXLO * ROWB], F8,
                                kind="ExternalOutput")
        dbg_tabloc = nc.dram_tensor("dbg_tabloc", [NPC, ROWB], F8,
                                    kind="ExternalOutput")

    groups = [list(range(NCORES))]
    # One sem per DMASW lane: tile_sem_assignment rotates Pool-engine DMA
    # insts over 8 global DMASW lanes in issue order and emits consumer
    # waits as (lane sem) >= 16 * lane_tick; the descriptor-baked sem must
    # therefore be unique per lane, mirroring that rotation exactly.
    gsem = [nc.alloc_semaphore(f"gsem{i}") for i in range(8)]

    with tile.TileContext(nc) as tc:
        with (
            tc.tile_pool(name="const", bufs=1) as constp,
            tc.tile_pool(name="obst", bufs=2) as obstp,
            tc.tile_pool(name="enc", bufs=2) as encp,
            tc.tile_pool(name="rows", bufs=3) as rowsp,
            tc.tile_pool(name="gath", bufs=KA + 1) as gathp,
            tc.tile_pool(name="small", bufs=3) as smallp,
            tc.tile_pool(name="rhs", bufs=2) as rhsp,
            tc.tile_pool(name="bt", bufs=2) as btp,
            tc.tile_pool(name="pe", bufs=2, space="PSUM") as pep,
            tc.tile_pool(name="pacc", bufs=2, space="PSUM") as paccp,
            tc.tile_pool(name="ptr", bufs=1, space="PSUM") as ptrp,
            tc.tile_pool(name="prod", bufs=1, space="PSUM") as prodp,
            tc.tile_pool(name="pers", bufs=2, space="PSUM") as persp,
        ):
            # ---------------- prologue ----------------
            iota_sb = constp.tile([P, P], BF, tag="iota")
            nc.sync.dma_start(iota_sb[:], iota_p[:, :])
            ident = constp.tile([P, P], F32, tag="ident")
            nc.sync.dma_start(ident[:], identf_p[:, :])
            iotac_sb = constp.tile([P, 1], BF, tag="iotac")
            nc.sync.dma_start(iotac_sb[:], iotac_p[:, :])
            il_sb = constp.tile([P, il_off[-1]], I16, tag="il")
            nc.scalar.dma_start(il_sb[:], il_p[:, :])
            ih_sb = constp.tile([P, ih_off[-1]], I16, tag="ih")
            nc.scalar.dma_start(ih_sb[:], ih_p[:, :])
            dpos_sb = constp.tile([P, dp_off_s[-1]], BF, tag="dpos")
            nc.scalar.dma_start(dpos_sb[:], dpos_p[:, :])
            er_sb0 = constp.tile([P, NT * NH], BF, tag="er_sb0")
            er_sb1 = constp.tile([P, NT * NH], BF, tag="er_sb1")
            er_sb = [er_sb0, er_sb1]

            W1_sb = []
            for k in range(2):
                t = constp.tile([P, HID], BF, tag=f"w1_{k}")
                nc.sync.dma_start(t[:], W1_p[k * P:(k + 1) * P, :])
                W1_sb.append(t)
            W2_sb = []
            for m in range(4):
                t = constp.tile([P, H], BF, tag=f"w2_{m}")
                nc.sync.dma_start(t[:], W2_p[m * P:(m + 1) * P, :])
                W2_sb.append(t)
            b1_sb = []
            for m in range(4):
                t = constp.tile([P, 1], F32, tag=f"b1_{m}")
                nc.sync.dma_start(t[:], b1_p[m * P:(m + 1) * P, :])
                b1_sb.append(t)
            b2_sb = constp.tile([P, 1], F32, tag="b2")
            nc.sync.dma_start(b2_sb[:], b2_p[:, :])
            Wg_sb, Wgal_sb, Wgar_sb = [], [], []
            for i in range(2):
                t = constp.tile([P, H], BF, tag=f"wg_{i}")
                nc.sync.dma_start(t[:], Wg_p[i][:, :])
                Wg_sb.append(t)
                t = constp.tile([P, NH], BF, tag=f"wgal_{i}")
                nc.sync.dma_start(t[:], Wgal_p[i][:, :])
                Wgal_sb.append(t)
                t = constp.tile([P, NH], BF, tag=f"wgar_{i}")
                nc.sync.dma_start(t[:], Wgar_p[i][:, :])
                Wgar_sb.append(t)

            # ------------- chunked AllGather -------------
            # The collective instruction is a dispatch-only trigger (ncfw
            # moves the data), so the first half of each table AllGather can
            # be issued as soon as its rows are written — overlapping the
            # encoder (AG1) / layer-1 (AG2) tails; only the second half sits
            # on the critical path.
            NCH = 1

            def ag_chunk(li, k):
                rows = NPC // NCH
                gr = NTOT // NCH
                nc.gpsimd.collective_compute(
                    "AllGather", OP.bypass, replica_groups=groups,
                    ins=[tab_loc[li][k * rows:(k + 1) * rows, :]],
                    outs=[tab_full[li][k * gr:(k + 1) * gr, :]])

            # ------------- gather prep machinery -------------
            gtiles = {}
            prep_ctr = [0]   # mirrors tile's DMASW lane rotation

            def issue_prep(li, S):
                tabf = tab_full[li]
                pkw0 = (dict(prepare_only=True,
                             sem=gsem[prep_ctr[0] % 8])
                        if USE_PREP else {})
                pkw1 = (dict(prepare_only=True,
                             sem=gsem[(prep_ctr[0] + 1) % 8])
                        if USE_PREP else {})
                prep_ctr[0] += 2
                cl, ch = NLO_S[S] // P, NHI_S[S] // P
                gl = gathp.tile([P, MAXLO * ROWB], F8, tag="glow")
                nc.gpsimd.dma_gather(
                    out_ap=gl[:, 0:cl * ROWB]
                        .rearrange("p (c e) -> p c e", e=ROWB),
                    in_ap=tabf[:, :],
                    idxs_ap=il_sb[:, il_off[S]:il_off[S + 1]],
                    num_idxs=NLO_S[S], num_idxs_reg=meta["nrl"][S],
                    elem_size=ROWB,
                    single_packet=False, queue_num=0, **pkw0)
                gh = gathp.tile([P, MAXHI * ROWB], F8, tag="ghigh")
                nc.gpsimd.dma_gather(
                    out_ap=gh[:, 0:ch * ROWB]
                        .rearrange("p (c e) -> p c e", e=ROWB),
                    in_ap=tabf[split:NTOT, :],
                    idxs_ap=ih_sb[:, ih_off[S]:ih_off[S + 1]],
                    num_idxs=NHI_S[S], num_idxs_reg=meta["nrh"][S],
                    elem_size=ROWB,
                    single_packet=False, queue_num=1 % NQ, **pkw1)
                gtiles[(li, S)] = (gl, gh)

            def fire_all():
                """Fire every untriggered prep (only current-layer preps may
                be pending when this is called)."""
                if not USE_PREP:
                    return
                for q in range(min(NQ, 2)):
                    nc.gpsimd.trigger_dma(count=None, queue_num=q)

            def table_products(zTb_chunk, row0, li):
                pr = prodp.tile([P, H + 2 * NH], F32, tag="pr")
                nc.tensor.matmul(pr[:, 0:H], lhsT=zTb_chunk, rhs=Wg_sb[li][:],
                                 start=True, stop=True)
                nc.tensor.matmul(pr[:, H:H + NH], lhsT=zTb_chunk,
                                 rhs=Wgal_sb[li][:], start=True, stop=True)
                nc.tensor.matmul(pr[:, H + NH:H + 2 * NH], lhsT=zTb_chunk,
                                 rhs=Wgar_sb[li][:], start=True, stop=True)
                rowt = rowsp.tile([P, H + NH], F8, tag="rowt")
                nc.vector.tensor_copy(rowt[:, 0:H], pr[:, 0:H])
                nc.vector.tensor_copy(rowt[:, H:H + NH], pr[:, H:H + NH])
                nc.sync.dma_start(
                    tab_loc[li][row0:row0 + P, 0:H + NH], rowt[:])
                D = row0 // P
                nc.vector.tensor_copy(er_sb[li][:, D * NH:(D + 1) * NH],
                                      pr[:, H + NH:H + 2 * NH])

            # Gather tiles are partially skipped (trailing -1 indices), so
            # zero them once: stale SBUF could hold fp8 NaN patterns that
            # would poison 0*NaN in the scatter matmul.
            for _ in range(KA + 1):
                t = gathp.tile([P, MAXLO * ROWB], F8, tag="glow")
                nc.gpsimd.memset(t[:], 0.0)
                t = gathp.tile([P, MAXHI * ROWB], F8, tag="ghigh")
                nc.gpsimd.memset(t[:], 0.0)

            # ----- layer-1 gather desc prep (hides under encoder+AG) -----
            if USE_PREP:
                for S in range(KA):
                    issue_prep(0, S)

            # ---------------- phase E: encoder ----------------
            for pt in range(NT // 2):
                n0 = pt * 2 * P
                obsT = []
                for k in range(2):
                    t = obstp.tile([P, 2 * P], BF, tag="obsT")
                    nc.sync.dma_start(t[:], obst_p[k * P:(k + 1) * P,
                                                   n0:n0 + 2 * P])
                    obsT.append(t)
                hT = []
                for m in range(4):
                    ph = pep.tile([P, 2 * P], F32, tag="pe")
                    for k in range(2):
                        nc.tensor.matmul(
                            ph[:], lhsT=W1_sb[k][:, m * P:(m + 1) * P],
                            rhs=obsT[k][:], start=(k == 0), stop=(k == 1))
                    h = encp.tile([P, 2 * P], BF, tag=f"h{m}")
                    nc.vector.tensor_scalar(
                        out=h[:], in0=ph[:], scalar1=b1_sb[m][:, 0:1],
                        scalar2=0.0, op0=OP.add, op1=OP.max)
                    hT.append(h)
                pz = pep.tile([P, 2 * P], F32, tag="pe")
                for m in range(4):
                    nc.tensor.matmul(pz[:], lhsT=W2_sb[m][:], rhs=hT[m][:],
                                     start=(m == 0), stop=(m == 3))
                z1T = encp.tile([P, 2 * P], F32, tag="z1T")
                nc.vector.tensor_scalar(
                    out=z1T[:], in0=pz[:], scalar1=b2_sb[:, 0:1],
                    scalar2=0.0, op0=OP.add, op1=OP.max)
                z1Tb = encp.tile([P, 2 * P], BF, tag="z1Tb")
                nc.vector.tensor_copy(z1Tb[:], z1T[:])
                for k in range(2):
                    ptr = ptrp.tile([P, P], F32, tag="ptr")
                    nc.tensor.transpose(ptr[:], z1T[:, k * P:(k + 1) * P],
                                        ident[:])
                    zrow = rowsp.tile([P, P], F32, tag="zrows")
                    nc.vector.tensor_copy(zrow[:], ptr[:])
                    nc.scalar.dma_start(
                        out_p[n0 + k * P:n0 + (k + 1) * P, 0:H], zrow[:])
                    table_products(z1Tb[:, k * P:(k + 1) * P], n0 + k * P, 0)
                if NCH == 2 and pt == NT // 4 - 1:
                    ag_chunk(0, 0)

            ag_chunk(0, NCH - 1)

            # ---------------- edge pass ----------------
            def epilogue(D, acc, out_col, build_next):
                zp = smallp.tile([P, NH], F32, tag="zp")
                nc.vector.tensor_scalar_add(zp[:], acc[:, H:TAB], 1e-9)
                zrec = smallp.tile([P, NH], F32, tag="zrec")
                nc.vector.reciprocal(zrec[:], zp[:])
                zo = rowsp.tile([P, H], F32, tag="zo")
                nc.vector.scalar_tensor_tensor(
                    out=zo[:].rearrange("p (h d) -> p h d", h=NH),
                    in0=acc[:, 0:H].rearrange("p (h d) -> p h d", h=NH),
                    scalar=0.0,
                    in1=zrec[:].unsqueeze(2).to_broadcast([P, NH, HD]),
                    op0=OP.max, op1=OP.mult)
                nc.scalar.dma_start(
                    out_p[D * P:(D + 1) * P, out_col:out_col + H], zo[:])
                if build_next:
                    pzt = ptrp.tile([P, P], F32, tag="ptr")
                    nc.tensor.transpose(pzt[:], zo[:], ident[:])
                    zTb = rowsp.tile([P, P], BF, tag="zTb")
                    nc.vector.tensor_copy(zTb[:], pzt[:])
                    table_products(zTb[:], D * P, 1)

            def edge_block(g, er_ps, er_off, dp_off, accs,
                           S, binmap, out_col, build_next, is_high):
                """One low/high block of a super: batched attn + per-tile mm."""
                n = len(binmap)
                g3 = g[:, 0:n * ROWB].rearrange("p (c e) -> p c e", e=ROWB)
                gfeat = g3[:, :, 0:H]
                gel = g3[:, :, H:H + NH]
                NMX = MAXHI if is_high else MAXLO
                e_t = smallp.tile([P, NMX * NH], F32,
                                  tag="e_th" if is_high else "e_tl")
                nc.vector.tensor_add(
                    e_t[:, 0:n * NH].rearrange("p (c e) -> p c e", e=NH),
                    gel,
                    er_ps[:, er_off * NH:(er_off + n) * NH]
                        .rearrange("p (c e) -> p c e", e=NH))
                lr = smallp.tile([P, NMX * NH], F32,
                                 tag="lrh" if is_high else "lrl")
                nc.vector.scalar_tensor_tensor(
                    out=lr[:, 0:n * NH], in0=e_t[:, 0:n * NH], scalar=0.2,
                    in1=e_t[:, 0:n * NH],
                    op0=OP.mult, op1=OP.max)
                rhs = rhsp.tile([P, NMX * TAB], BF,
                                tag="rhsh" if is_high else "rhsl")
                r3 = rhs[:, 0:n * TAB].rearrange("p (c e) -> p c e", e=TAB)
                nc.scalar.activation(
                    r3[:, :, H:TAB],
                    lr[:, 0:n * NH].rearrange("p (c e) -> p c e", e=NH),
                    AF.Exp)
                bt = btp.tile([P, NMX * P], BF,
                              tag="bth" if is_high else "btl")
                nc.vector.tensor_tensor(
                    out=bt[:, 0:n * P].rearrange("p (c e) -> p c e", e=P),
                    in0=dpos_sb[:, dp_off:dp_off + n].unsqueeze(2)
                        .to_broadcast([P, n, P]),
                    in1=iota_sb[:].unsqueeze(1).to_broadcast([P, n, P]),
                    op=OP.is_equal)
                nc.vector.tensor_tensor(
                    out=r3[:, :, 0:H].rearrange("p c (h d) -> p c h d", h=NH),
                    in0=gfeat.rearrange("p c (h d) -> p c h d", h=NH),
                    in1=r3[:, :, H:TAB].unsqueeze(3)
                        .to_broadcast([P, n, NH, HD]),
                    op=OP.mult)
                for c in range(n):
                    b = binmap[c]
                    first = c == 0 or binmap[c - 1] != b
                    last = c == n - 1 or binmap[c + 1] != b
                    D = S * SUP + b
                    if not is_high and first:
                        acc_new = paccp.tile([P, TAB], F32, tag="acc")
                        accs[b] = acc_new
                    nc.tensor.matmul(
                        accs[b][:], lhsT=bt[:, c * P:(c + 1) * P],
                        rhs=r3[:, c, :],
                        start=(not is_high and first),
                        stop=(is_high and last))
                    if is_high and last:
                        epilogue(D, accs[b], out_col, build_next)

            def edge_pass(li, out_col, build_next):
                accs = [None] * SUP
                # Fence: the first trigger must not fire before the AllGather
                # has written tab_full[li].  An SP probe read of the table
                # picks up the collective-completion wait; a Pool-engine copy
                # of the probe then blocks the GpSimd engine (and hence the
                # triggers behind it) until that data has landed.
                probe = smallp.tile([P, 2], F8, tag="probe")
                nc.sync.dma_start(probe[:], tab_full[li][0:P, 0:2])
                fence = smallp.tile([P, 2], F8, tag="fence")
                nc.gpsimd.tensor_copy(fence[:], probe[:])
                fire_all()
                for S in range(NSUP):
                    nst = NST_S[S]
                    lmap = ([0] * LS[S][0]) + ([1] * LS[S][1])
                    hmap = ([0] * HS[S][0]) + ([1] * HS[S][1])
                    dprow = btp.tile([P, MAXNST * P], BF, tag="dprow")
                    nc.sync.dma_start(
                        dprow[:, 0:nst * P],
                        dprow_p[0:1, dr_off_s[S]:dr_off_s[S + 1]]
                        .to_broadcast([P, nst * P]))
                    ball = btp.tile([P, MAXNST * P], BF, tag="ball")
                    nc.vector.tensor_tensor(
                        out=ball[:, 0:nst * P]
                            .rearrange("p (c e) -> p c e", e=P),
                        in0=iotac_sb[:, 0:1].unsqueeze(2)
                            .to_broadcast([P, nst, P]),
                        in1=dprow[:, 0:nst * P]
                            .rearrange("p (c e) -> p c e", e=P),
                        op=OP.is_equal)
                    er_ps = persp.tile([P, MAXNST * NH], F32, tag="er_ps")
                    for c, b in enumerate(lmap + hmap):
                        D = S * SUP + b
                        nc.tensor.matmul(
                            er_ps[:, c * NH:(c + 1) * NH],
                            lhsT=ball[:, c * P:(c + 1) * P],
                            rhs=er_sb[li][:, D * NH:(D + 1) * NH],
                            start=True, stop=True)

                    if not USE_PREP:
                        issue_prep(li, S)
                    gl, gh = gtiles.pop((li, S))
                    if DBG and li == 1 and S == 0:
                        nc.sync.dma_start(dbg_gl[:, :], gl[:])
                        nc.sync.dma_start(dbg_tab[:, :], tab_full[1][:, :])
                    edge_block(gl[:], er_ps, 0, dp_off_s[S], accs,
                               S, lmap, out_col, build_next, False)
                    edge_block(gh[:], er_ps, len(lmap),
                               dp_off_s[S] + len(lmap),
                               accs, S, hmap, out_col, build_next, True)

                    # prep K_AHEAD supers forward (flows into next layer);
                    # current-layer preps are fired immediately, next-layer
                    # preps stay untriggered until the next edge_pass's
                    # post-AllGather fire_all.
                    if USE_PREP:
                        nS = S + KA
                        if nS < NSUP:
                            issue_prep(li, nS)
                            fire_all()
                        elif li == 0 and nS - NSUP < min(KA, NSUP):
                            issue_prep(1, nS - NSUP)
                    if (NCH == 2 and li == 0 and build_next
                            and S == NSUP // 2 - 1):
                        ag_chunk(1, 0)

            edge_pass(0, H, True)

            if DBG:
                nc.sync.dma_start(dbg_tabloc[:, :], tab_loc[1][:, :])
            ag_chunk(1, NCH - 1)

            edge_pass(1, 2 * H, False)

    if USE_PREP:
        # Tile emits data-consumer waits on its own per-lane DMASW{k} sems
        # (>= 16 * lane tick) but leaves the caller's sem= in the prep's
        # OnUpdate[0] — which is what the DMA descriptors actually bump.
        # Point OnUpdate[0] at the lane sem so the waits mean something.
        def _walk():
            for f in nc.m.functions:
                for bb in f.blocks:
                    for i in bb.instructions:
                        yield i
        lane_sem_id = {}
        for i in _walk():
            si = i.sync_info
            if not si:
                continue
            for x in list(si.on_wait or []) + list(si.on_update or []):
                nm = x.ant_name
                if nm.startswith("DMASW"):
                    lane_sem_id[int(nm[5:].split("_")[0])] = x.id
        fixed = 0
        for i in _walk():
            if (type(i).__name__ == "InstDMAGatherAnt"
                    and getattr(i, "gen_mode", 0) == 1):
                lane = i.bass_scheduled_proc - 11
                if lane in lane_sem_id:
                    i.sync_info.on_update[0].id = lane_sem_id[lane]
                    fixed += 1
        assert fixed == 0 or len(lane_sem_id) > 0

    nc.compile()
    return nc


# ----------------------------------------------------------------------------
# Driver
# ----------------------------------------------------------------------------

def _make_blockdiag(a):
    bd = np.zeros((H, NH), np.float32)
    for h in range(NH):
        bd[h * HD:(h + 1) * HD, h] = a[h]
    return bd


def run_gnn(inputs, n_tiles_per_core=52, trace=False):
    import ml_dtypes
    bf16 = ml_dtypes.bfloat16

    t_start = time.time()
    obs = np.asarray(inputs["obs"], np.float32)
    src = np.asarray(inputs["src"], np.int64)
    dst = np.asarray(inputs["dst"], np.int64)
    N = obs.shape[0]

    NTOT_ = NCORES * n_tiles_per_core * P
    split = min(SPLIT, NTOT_ // 2)
    prep = _host_prepare(src, dst, n_tiles_per_core, split)
    NT = n_tiles_per_core
    TL, TH, NPC, NTOT = prep["TL"], prep["TH"], prep["NPC"], prep["NTOT"]
    perm = prep["perm"]

    al1bd = _make_blockdiag(np.asarray(inputs["al1"], np.float32))
    ar1bd = _make_blockdiag(np.asarray(inputs["ar1"], np.float32))
    al2bd = _make_blockdiag(np.asarray(inputs["al2"], np.float32))
    ar2bd = _make_blockdiag(np.asarray(inputs["ar2"], np.float32))
    Wg1 = np.asarray(inputs["Wg1"], np.float32)
    Wg2 = np.asarray(inputs["Wg2"], np.float32)
    shared = {
        "w1": np.asarray(inputs["W1"], np.float32).astype(bf16),
        "b1": np.asarray(inputs["b1"], np.float32).reshape(HID, 1),
        "w2": np.asarray(inputs["W2"], np.float32).astype(bf16),
        "b2": np.asarray(inputs["b2"], np.float32).reshape(H, 1),
        "wg1": Wg1.astype(bf16), "wg2": Wg2.astype(bf16),
        "wgal1": (Wg1 @ al1bd).astype(bf16),
        "wgar1": (Wg1 @ ar1bd).astype(bf16),
        "wgal2": (Wg2 @ al2bd).astype(bf16),
        "wgar2": (Wg2 @ ar2bd).astype(bf16),
        "iota": np.tile(np.arange(P, dtype=np.float32)[None, :],
                        (P, 1)).astype(bf16),
        "identf": np.eye(P, dtype=np.float32),
        "iotac": np.arange(P, dtype=np.float32).reshape(P, 1).astype(bf16),
    }

    obs_pad = np.zeros((NTOT, OBS_D), np.float32)
    obs_pad[:N] = obs
    obs_perm = obs_pad[perm]

    in_maps = []
    for c in range(NCORES):
        m = dict(shared)
        m["obst"] = np.ascontiguousarray(
            obs_perm[c * NPC:(c + 1) * NPC].T).astype(bf16)
        m["idxlow"] = prep["idxlow"][c]
        m["idxhigh"] = prep["idxhigh"][c]
        m["dprow"] = prep["dprow"][c]
        m["dposslab"] = prep["dpos"][c]
        in_maps.append(m)

    t_prep = time.time()
    nc = _build_program(NT, TL, TH, split, prep["meta"])
    t_build = time.time()

    from concourse.bass_utils import run_bass_kernel_spmd
    if trace:
        _ensure_ntff_hook()
    res = run_bass_kernel_spmd(nc, in_maps, core_ids=list(range(NCORES)),
                               trace=trace)
    t_run = time.time()

    full = np.concatenate([res.results[c]["out"] for c in range(NCORES)],
                          axis=0)
    out = np.empty((N, 3 * H), np.float32)
    keep = perm < N
    out[perm[keep]] = full[keep]

    LAST_INFO.clear()
    if os.environ.get("GNN_DBG"):
        LAST_INFO["dbg"] = {k: [res.results[c].get(k) for c in range(NCORES)]
                            for k in ("dbg_tab", "dbg_gl", "dbg_tabloc")}
        LAST_INFO["prep"] = prep
    LAST_INFO.update(dict(
        exec_time_ns=res.exec_time_ns, TL=TL, TH=TH,
        binload_max=int(prep["binload"].max()),
        t_prep=t_prep - t_start, t_build=t_build - t_prep,
        t_run=t_run - t_build,
        profile_json=getattr(res, "profile_json", None),
    ))
    return out


def kernel(**inputs):
    return run_gnn(inputs, n_tiles_per_core=52,
                   trace=bool(os.environ.get("GNN_TRACE")))
